# revision 1
# baseline (speedup 1.0000x reference)
"""Causal self-attention (GQA + RoPE) Trainium2 Bass kernel, 8-core SPMD.

Problem shapes (hardcoded): B=2, S=2048, D=1024, NH=16 q-heads, KVH=4
kv-heads, HD=64, RoPE base 10000, fp32 I/O.

Sharding (batch x kv-group): core c -> batch b = c//4, kv-group g = c%4.
Each kv-group owns one kv head and its 4 q heads (GQA repeat=4), so the
whole causal attention for those heads is local to the core. Each core
computes the partial output projection y_g @ Wo[g-block]; the host sums
the 4 partials per batch ("gather/unshard").

Per-core device kernel:
  inputs : xT [1024, 2048] (x[b] transposed on host), wq [1024, 256],
           wkv [1024, 128] (Wk_g ++ Wv_g), wo [256, 1024]
  output : out [2048, 1024] partial

Everything on-chip is kept transposed ([head_dim, seq]) so QK^T and PV
need no transposes: scores_T[k, q] = k_T.T @ q_T with the head dim as
the PE contraction axis, and y_T[d, q] = v_chunk.T @ expP_T. Softmax
runs along the PARTITION axis for free: v is augmented with a ones
column, so the PV accumulation drops the softmax denominator into row
64 of y_aug; division = reciprocal + K=1 broadcast matmul. Causality is
handled at 128-granularity by skipping fully-masked key chunks plus one
128x128 triangular mask multiply per diagonal block.

Projections / out-proj run in f32r (tf32-like); attention matmuls in
bf16 (pipelined PE weight loads). Measured vs the fp32 reference:
rel err ~1.9e-3, HW exec ~230 us.
"""
import numpy as np
from contextlib import ExitStack

import concourse.bass as bass
import concourse.tile as tile
from concourse import bacc, mybir
from concourse.bass_utils import run_bass_kernel_spmd

F32 = mybir.dt.float32
F32R = mybir.dt.float32r
BF16 = mybir.dt.bfloat16
AF = mybir.ActivationFunctionType

B, S, D = 2, 2048, 1024
NH, KVH, HD = 16, 4, 64
N_CORES = 8
SCALE = HD ** -0.5  # 0.125

DT_ATTN = BF16  # dtype for q'/k/v/expP and the scores+PV matmuls

_CACHE = {}


def _rope_tables():
    half = HD // 2
    inv_freq = (1.0 / (10000.0 ** (np.arange(half, dtype=np.float32) / half))
                ).astype(np.float32)
    t = np.arange(S, dtype=np.float32)
    freqs = np.outer(t, inv_freq).astype(np.float32)      # [S, 32]
    emb = np.concatenate([freqs, freqs], axis=1)          # [S, 64]
    cos_T = np.cos(emb).T.astype(np.float32)              # [64, S]
    sin_T = np.sin(emb).T.astype(np.float32)
    sin_n = sin_T.copy()
    sin_n[:half] *= -1.0                                  # sign for rotate_half
    cos4 = np.tile(cos_T, (2, 1))                         # [128, S] (2 heads)
    sin4 = np.tile(sin_n, (2, 1))
    return cos4, sin4


def _build_kernel():
    nc = bacc.Bacc("TRN2", target_bir_lowering=False, debug=False,
                   num_devices=N_CORES)

    xT_ap = nc.dram_tensor("xT", [D, S], F32, kind="ExternalInput").ap()
    wq_ap = nc.dram_tensor("wq", [D, 256], F32, kind="ExternalInput").ap()
    wkv_ap = nc.dram_tensor("wkv", [D, 128], F32, kind="ExternalInput").ap()
    wo_ap = nc.dram_tensor("wo", [256, D], F32, kind="ExternalInput").ap()
    out_ap = nc.dram_tensor("out", [S, D], F32, kind="ExternalOutput").ap()

    cos4_np, sin4_np = _rope_tables()
    cos4_d = nc.inline_tensor(cos4_np, name="cos4").ap()
    sin4_d = nc.inline_tensor(sin4_np, name="sin4").ap()
    tri_np = (np.arange(128)[None, :] >= np.arange(128)[:, None]
              ).astype(np.float32)                         # [k, q] keep q>=k
    tri_d = nc.inline_tensor(tri_np, name="tri").ap()
    ones64_d = nc.inline_tensor(np.ones((1, 64), np.float32), name="ones64").ap()
    ones16_d = nc.inline_tensor(np.ones((128, 16), np.float32), name="ones16").ap()
    id_np = np.zeros((128, 64), np.float32)
    id_np[64:128] = np.eye(64, dtype=np.float32)
    id_d = nc.inline_tensor(id_np, name="id64").ap()

    with tile.TileContext(nc) as tc, ExitStack() as top:
        # ---- constants + persistent sbuf tiles -------------------------
        const = top.enter_context(tc.tile_pool(name="const", bufs=1))
        cos4 = const.tile([128, S], DT_ATTN, tag="cos4")
        sin4 = const.tile([128, S], DT_ATTN, tag="sin4")
        tri = const.tile([128, 128], DT_ATTN, tag="tri")
        ones_r = const.tile([1, 64], F32R, tag="ones_r")
        id64 = const.tile([128, 64], DT_ATTN, tag="id64")
        nc.gpsimd.dma_start(cos4[:], cos4_d[:])
        nc.gpsimd.dma_start(sin4[:], sin4_d[:])
        nc.gpsimd.dma_start(tri[:], tri_d[:])
        nc.gpsimd.dma_start(ones_r[:], ones64_d[:])
        nc.gpsimd.dma_start(id64[:], id_d[:])

        wpool = top.enter_context(tc.tile_pool(name="w", bufs=1))
        wq_sb = wpool.tile([128, 8 * 256], F32R, tag="wq")
        wkv_sb = wpool.tile([128, 8 * 128], F32R, tag="wkv")
        wo_sb = wpool.tile([128, 2 * 1024], F32R, tag="wo")
        nc.gpsimd.dma_start(wkv_sb[:].rearrange("p (kc m) -> p kc m", kc=8),
                            wkv_ap.rearrange("(kc p) m -> p kc m", p=128))
        nc.gpsimd.dma_start(wq_sb[:].rearrange("p (kc m) -> p kc m", kc=8),
                            wq_ap.rearrange("(kc p) m -> p kc m", p=128))
        nc.gpsimd.dma_start(wo_sb[:].rearrange("p (c n) -> p c n", c=2),
                            wo_ap.rearrange("(c p) n -> p c n", p=128))

        # q'/k/v results live through the whole kernel
        act = top.enter_context(tc.tile_pool(name="acts", bufs=1))
        qp = [act.tile([128, S], DT_ATTN, tag=f"qp{i}", name=f"qp{i}")
              for i in range(2)]
        kk = act.tile([128, S], DT_ATTN, tag="kk")
        v_all = act.tile([128, 16 * 80], DT_ATTN, tag="v_all")
        Y = [act.tile([128, S], F32R, tag=f"Y{i}", name=f"Y{i}") for i in range(2)]

        # ---- phase 1+2+3: projections, RoPE, v_aug ---------------------
        with ExitStack() as ph1:
            raw = ph1.enter_context(tc.tile_pool(name="raw", bufs=1))
            qraw = [raw.tile([128, S], DT_ATTN, tag=f"qraw{i}", name=f"qraw{i}")
                    for i in range(2)]
            kvraw = raw.tile([128, S], DT_ATTN, tag="kvraw")
            shp = ph1.enter_context(tc.tile_pool(name="shift", bufs=1))
            qsh = [shp.tile([128, S], DT_ATTN, tag=f"qsh{i}", name=f"qsh{i}")
                   for i in range(2)]
            ksh = shp.tile([64, S], DT_ATTN, tag="ksh")

            phx = ph1.enter_context(ExitStack())
            xpool = phx.enter_context(tc.tile_pool(name="xT", bufs=1))
            xT = []
            for kc in range(8):
                t = xpool.tile([128, S], F32R, tag=f"x{kc}", name=f"x{kc}")
                nc.gpsimd.dma_start(t[:], xT_ap[kc * 128:(kc + 1) * 128, :])
                xT.append(t)

            pps = phx.enter_context(tc.tile_pool(name="pj", bufs=8, space="PSUM"))
            with nc.named_scope("proj"):
                # kc-outer: each xT chunk is consumed right after its DMA
                # lands; kv + q-mt0 accumulate in pass 1 (8 banks), q-mt1
                # in pass 2.
                kv_ps = [pps.tile([128, 512], F32, tag="pj", name=f"kvps{nt}")
                         for nt in range(4)]
                q0_ps = [pps.tile([128, 512], F32, tag="pj", name=f"q0ps{nt}")
                         for nt in range(4)]
                for kc in range(8):
                    for nt in range(4):
                        nc.tensor.matmul(
                            kv_ps[nt][:], wkv_sb[:, kc * 128:(kc + 1) * 128],
                            xT[kc][:, nt * 512:(nt + 1) * 512],
                            start=(kc == 0), stop=(kc == 7))
                        nc.tensor.matmul(
                            q0_ps[nt][:], wq_sb[:, kc * 256:kc * 256 + 128],
                            xT[kc][:, nt * 512:(nt + 1) * 512],
                            start=(kc == 0), stop=(kc == 7))
                for nt in range(4):
                    nc.vector.tensor_copy(kvraw[:, nt * 512:(nt + 1) * 512],
                                          kv_ps[nt][:])
                q1_ps = [pps.tile([128, 512], F32, tag="pj", name=f"q1ps{nt}")
                         for nt in range(4)]
                for kc in range(8):
                    for nt in range(4):
                        nc.tensor.matmul(
                            q1_ps[nt][:], wq_sb[:, kc * 256 + 128:kc * 256 + 256],
                            xT[kc][:, nt * 512:(nt + 1) * 512],
                            start=(kc == 0), stop=(kc == 7))
                for nt in range(4):
                    nc.vector.tensor_copy(qraw[0][:, nt * 512:(nt + 1) * 512],
                                          q0_ps[nt][:])
                    nc.vector.tensor_copy(qraw[1][:, nt * 512:(nt + 1) * 512],
                                          q1_ps[nt][:])
            phx.close()

            with nc.named_scope("rope"):
                nc.sync.dma_start(ksh[0:32, :], kvraw[32:64, :])
                nc.sync.dma_start(ksh[32:64, :], kvraw[0:32, :])
                nc.vector.tensor_mul(kk[0:64, :], kvraw[0:64, :], cos4[0:64, :])
                nc.vector.tensor_mul(ksh[:], ksh[:], sin4[0:64, :])
                nc.vector.tensor_add(kk[0:64, :], kk[0:64, :], ksh[:])
                nc.sync.dma_start(kk[64:128, :], kk[0:64, :])
                for i in range(2):
                    for h in range(2):
                        base = h * 64
                        nc.sync.dma_start(qsh[i][base:base + 32, :],
                                          qraw[i][base + 32:base + 64, :])
                        nc.sync.dma_start(qsh[i][base + 32:base + 64, :],
                                          qraw[i][base:base + 32, :])
                    nc.vector.tensor_mul(qp[i][:], qraw[i][:], cos4[:])
                    nc.vector.tensor_mul(qsh[i][:], qsh[i][:], sin4[:])
                    nc.vector.tensor_add(qp[i][:], qp[i][:], qsh[i][:])

            with nc.named_scope("vprep"), ExitStack() as ph3:
                vps = ph3.enter_context(tc.tile_pool(name="vt", bufs=2,
                                                     space="PSUM"))
                ones_cols = v_all[:].rearrange("p (s c) -> p s c", c=80)[:, :, 64]
                nc.gpsimd.dma_start(ones_cols, ones16_d[:])
                for st in range(16):
                    tp = vps.tile([128, 64], DT_ATTN)
                    nc.tensor.transpose(
                        tp[:], kvraw[64:128, st * 128:(st + 1) * 128],
                        id64[64:128, :])
                    nc.vector.tensor_copy(v_all[:, st * 80:st * 80 + 64], tp[:])

        # ---- attention + interleaved out-projection --------------------
        spool = top.enter_context(tc.tile_pool(name="sc", bufs=2, space="PSUM"))
        epool = top.enter_context(tc.tile_pool(name="ex", bufs=3))
        ypool = top.enter_context(tc.tile_pool(name="yps", bufs=3, space="PSUM"))
        opool = top.enter_context(tc.tile_pool(name="op", bufs=1, space="PSUM"))
        dpool = top.enter_context(tc.tile_pool(name="div", bufs=2))
        osb = top.enter_context(tc.tile_pool(name="osb", bufs=3))
        oev = [0]

        with nc.named_scope("attn"):
            for qt in range(4):
                for pair in range(2):
                    nkc = 4 * qt + 4
                    y_ps = [ypool.tile([65, 512], F32, tag="y",
                                       name=f"y{pair}{qt}{_h}") for _h in range(2)]
                    for G in range(nkc // 2):
                        for hl in range(2):
                            hb = hl * 64
                            sc = spool.tile([128, 1024], F32, tag="sc",
                                            name=f"sc{pair}{qt}{G}{hl}")
                            for ci in range(2):
                                kc = 2 * G + ci
                                nc.tensor.matmul(
                                    sc[:, ci * 512:(ci + 1) * 512],
                                    kk[hb:hb + 64, kc * 128:(kc + 1) * 128],
                                    qp[pair][hb:hb + 64, qt * 512:(qt + 1) * 512],
                                    start=True, stop=True)
                            ex = epool.tile([128, 1024], DT_ATTN, tag="ex",
                                            name=f"ex{pair}{qt}{G}{hl}")
                            nc.scalar.activation(ex[:], sc[:], AF.Exp, scale=SCALE)
                            for ci in range(2):
                                kc = 2 * G + ci
                                j = kc - 4 * qt
                                off = 0
                                if 0 <= j < 4:  # diagonal chunk
                                    off = j * 128
                                    mslice = ex[:, ci * 512 + off:ci * 512 + off + 128]
                                    nc.vector.tensor_mul(mslice, mslice, tri[:])
                                nc.tensor.matmul(
                                    y_ps[hl][:, off:512],
                                    v_all[:, kc * 80:kc * 80 + 65],
                                    ex[:, ci * 512 + off:(ci + 1) * 512],
                                    start=(kc == 0), stop=(kc == nkc - 1))
                    # division for both heads of this (pair, qt): the
                    # denominators sit in row 64 of y_ps (v_aug ones col);
                    # reciprocal + K=1 broadcast matmul, then one multiply
                    # per head and a DMA into the stacked Y tile.
                    ysb = dpool.tile([64, 1024], F32, tag="ysb")
                    dn = dpool.tile([1, 1024], F32, tag="dn")
                    nc.vector.tensor_copy(ysb[:, 0:512], y_ps[0][0:64, :])
                    nc.vector.tensor_copy(ysb[:, 512:1024], y_ps[1][0:64, :])
                    nc.scalar.copy(dn[:, 0:512], y_ps[0][64:65, :])
                    nc.scalar.copy(dn[:, 512:1024], y_ps[1][64:65, :])
                    recf = dpool.tile([1, 1024], F32, tag="recf")
                    nc.vector.reciprocal_approx_fast(recf[:], dn[:])
                    recr = dpool.tile([1, 1024], F32R, tag="recr")
                    nc.vector.tensor_copy(recr[:], recf[:])
                    ytmp = dpool.tile([64, 1024], F32R, tag="ytmp")
                    for hl in range(2):
                        bc = opool.tile([64, 512], F32, tag="po",
                                        name=f"bc{pair}{qt}{hl}")
                        nc.tensor.matmul(bc[:], ones_r[:],
                                         recr[:, hl * 512:(hl + 1) * 512],
                                         start=True, stop=True)
                        nc.vector.tensor_mul(ytmp[:, hl * 512:(hl + 1) * 512],
                                             ysb[:, hl * 512:(hl + 1) * 512],
                                             bc[:])
                        nc.sync.dma_start(
                            Y[pair][hl * 64:hl * 64 + 64, qt * 512:(qt + 1) * 512],
                            ytmp[:, hl * 512:(hl + 1) * 512])
                # out-projection for this qt's four s-tiles
                with nc.named_scope("outproj"):
                    for st in range(4 * qt, 4 * qt + 4):
                        for nt in range(2):
                            po = opool.tile([128, 512], F32, tag="po")
                            for cc in range(2):
                                nc.tensor.matmul(
                                    po[:],
                                    Y[cc][:, st * 128:(st + 1) * 128],
                                    wo_sb[:, cc * 1024 + nt * 512:cc * 1024 + (nt + 1) * 512],
                                    start=(cc == 0), stop=(cc == 1))
                            ot = osb.tile([128, 512], F32, tag="ot")
                            if oev[0] % 4 != 3:
                                nc.vector.tensor_copy(ot[:], po[:])
                            else:
                                nc.scalar.copy(ot[:], po[:])
                            oev[0] += 1
                            nc.sync.dma_start(
                                out_ap[st * 128:(st + 1) * 128, nt * 512:(nt + 1) * 512],
                                ot[:])

    nc.compile()
    return nc


def _shard_inputs(x, Wq, Wk, Wv, Wo):
    in_maps = []
    for c in range(N_CORES):
        b, g = divmod(c, 4)
        in_maps.append({
            "xT": np.ascontiguousarray(x[b].T).astype(np.float32),
            "wq": np.ascontiguousarray(
                Wq[:, g * 256:(g + 1) * 256]).astype(np.float32),
            "wkv": np.ascontiguousarray(np.concatenate(
                [Wk[:, g * 64:(g + 1) * 64], Wv[:, g * 64:(g + 1) * 64]],
                axis=1)).astype(np.float32),
            "wo": np.ascontiguousarray(
                Wo[g * 256:(g + 1) * 256, :]).astype(np.float32),
        })
    return in_maps


def kernel(x, Wq, Wk, Wv, Wo):
    x = np.asarray(x, dtype=np.float32)
    Wq = np.asarray(Wq, dtype=np.float32)
    Wk = np.asarray(Wk, dtype=np.float32)
    Wv = np.asarray(Wv, dtype=np.float32)
    Wo = np.asarray(Wo, dtype=np.float32)
    assert x.shape == (B, S, D), x.shape

    if "nc" not in _CACHE:
        _CACHE["nc"] = _build_kernel()
    nc = _CACHE["nc"]

    in_maps = _shard_inputs(x, Wq, Wk, Wv, Wo)
    res = run_bass_kernel_spmd(nc, in_maps, list(range(N_CORES)))

    out = np.zeros((B, S, D), dtype=np.float32)
    for c in range(N_CORES):
        out[c // 4] += res.results[c]["out"]
    return out

